# revision 12
# baseline (speedup 1.0000x reference)
"""Gated FFN (top-1 tile routing) on 8 trn2 NeuronCores.

Math (forward, eval mode):
    logits = x @ gate_w + gate_b            # [N, 8]
    gate   = one_hot(argmax(logits))        # straight-through == one-hot fwd
    hidden = relu(x @ up_w[t_n] + up_b[t_n])
    out    = hidden @ sum_t(down_w[t]) + sum_t(down_b[t])

Sharding: expert-parallel. Core t receives the tokens routed to tile t
(gathered + padded to CAP on host), up_w[t], and the tile-summed down
matrix Wd (replicated). Routing/gather/scatter and the tiny gate matmul
run on host; the two big matmuls run on device.
"""

import os

import numpy as np

B, T, D, TILES = 2, 1024, 1024, 8
DFF = 4 * D
N = B * T
P = 128

# matmul dtype mode: "f32" (safe), "f32r" (fp32 data, fast PE path), "bf16"
MODE = os.environ.get("GFFN_MODE", "f32r")

_built = {}


def _build(cap: int, mode: str):
    import concourse.bacc as bacc
    import concourse.mybir as mybir
    from concourse import tile

    dt = mybir.dt
    io_dt = {"bf16": dt.bfloat16, "f32r": dt.float32r, "f32": dt.float32}[mode]
    AF = mybir.ActivationFunctionType

    def mm(ap):
        return ap

    nc = bacc.Bacc(target_bir_lowering=False, num_swdge_queues=4)
    xT = nc.declare_dram_parameter("xT", [D, cap], io_dt, isOutput=False)
    upw = nc.declare_dram_parameter("upw", [D, DFF], io_dt, isOutput=False)
    upb = nc.declare_dram_parameter("upb", [P, DFF // P], dt.float32, isOutput=False)
    wd = nc.declare_dram_parameter("wd", [DFF, D], io_dt, isOutput=False)
    bd = nc.declare_dram_parameter("bd", [P, D // P], dt.float32, isOutput=False)
    outT = nc.declare_dram_parameter("outT", [D, cap], dt.float32, isOutput=True)

    KD = D // P        # 8 contraction blocks for up
    MF = DFF // P      # 32 dff blocks
    G = 4              # psum banks used per group

    with tile.TileContext(nc) as tc:
        with (
            tc.tile_pool(name="cst", bufs=2) as cst,
            tc.tile_pool(name="xp", bufs=KD) as xp,
            tc.tile_pool(name="wp", bufs=8) as wp,
            tc.tile_pool(name="hp", bufs=MF) as hp,
            tc.tile_pool(name="wdp", bufs=8) as wdp,
            tc.tile_pool(name="op", bufs=4) as op,
            tc.tile_pool(name="ps", bufs=8, space="PSUM") as ps,
        ):
            upb_t = cst.tile([P, MF], dt.float32, tag="upb")
            nc.sync.dma_start(out=upb_t[:], in_=upb[:])
            bd_t = cst.tile([P, D // P], dt.float32, tag="bd")
            nc.sync.dma_start(out=bd_t[:], in_=bd[:])

            xts = []
            for k in range(KD):
                t = xp.tile([P, cap], io_dt, tag="x")
                nc.sync.dma_start(out=t[:], in_=xT[k * P:(k + 1) * P, :])
                xts.append(t)

            # up projection: hT[m] = relu(up_w[:, mP:(m+1)P].T @ xT + up_b[m])
            hts = []
            for mg in range(MF // G):
                psh = [ps.tile([P, cap], dt.float32, tag="ps", name="psb") for _ in range(G)]
                c0 = mg * G * P
                for k in range(KD):
                    wt = wp.tile([P, G * P], io_dt, tag="w")
                    nc.sync.dma_start(
                        out=wt[:], in_=upw[k * P:(k + 1) * P, c0:c0 + G * P]
                    )
                    for j in range(G):
                        nc.tensor.matmul(
                            psh[j][:],
                            mm(wt[:, j * P:(j + 1) * P]),
                            mm(xts[k][:]),
                            start=(k == 0),
                            stop=(k == KD - 1),
                        )
                for j in range(G):
                    m = mg * G + j
                    ht = hp.tile([P, cap], io_dt, tag="h")
                    nc.scalar.activation(
                        ht[:], psh[j][:], AF.Relu, bias=upb_t[:, m:m + 1]
                    )
                    hts.append(ht)

            # down projection: outT[n] = sum_m wd[mP:(m+1)P, nP:(n+1)P].T @ hT[m] + bd[n]
            for ng in range((D // P) // G):
                pso = [ps.tile([P, cap], dt.float32, tag="ps", name="psb") for _ in range(G)]
                c0 = ng * G * P
                for m in range(MF):
                    wdt = wdp.tile([P, G * P], io_dt, tag="wd")
                    nc.sync.dma_start(
                        out=wdt[:], in_=wd[m * P:(m + 1) * P, c0:c0 + G * P]
                    )
                    for j in range(G):
                        nc.tensor.matmul(
                            pso[j][:],
                            mm(wdt[:, j * P:(j + 1) * P]),
                            mm(hts[m][:]),
                            start=(m == 0),
                            stop=(m == MF - 1),
                        )
                for j in range(G):
                    n = ng * G + j
                    ot = op.tile([P, cap], dt.float32, tag="o")
                    nc.scalar.activation(
                        ot[:], pso[j][:], AF.Identity, bias=bd_t[:, n:n + 1]
                    )
                    nc.sync.dma_start(
                        out=outT[n * P:(n + 1) * P, :], in_=ot[:]
                    )
    nc.finalize()
    return nc


def _get_nc(cap: int, mode: str):
    key = (cap, mode)
    if key not in _built:
        _built[key] = _build(cap, mode)
    return _built[key]


_execs = {}


def _build_exec(cap: int, mode: str):
    """Compile the SPMD kernel to a reusable sharded jax callable.

    Adapted from concourse.bass2jax.run_bass_via_pjrt (no donation, so the
    callable can be re-invoked for steady-state timing).
    """
    import jax
    from jax.experimental.shard_map import shard_map
    from jax.sharding import Mesh, PartitionSpec

    import concourse.mybir as mybir
    from concourse import bass2jax

    nc = _get_nc(cap, mode)
    bass2jax.install_neuronx_cc_hook()

    partition_name = nc.partition_id_tensor.name if nc.partition_id_tensor else None
    in_names, out_names, out_avals, zero_outs = [], [], [], []
    for alloc in nc.m.functions[0].allocations:
        if not isinstance(alloc, mybir.MemoryLocationSet):
            continue
        name = alloc.memorylocations[0].name
        if alloc.kind == "ExternalInput":
            if name != partition_name:
                in_names.append(name)
        elif alloc.kind == "ExternalOutput":
            shape = list(alloc.tensor_shape)
            np_dt = mybir.dt.np(alloc.dtype)
            out_names.append(name)
            out_avals.append(jax.core.ShapedArray(shape, np_dt))
            zero_outs.append(np.zeros(shape, np_dt))

    n_params = len(in_names)
    all_in_names = in_names + out_names
    if partition_name is not None:
        all_in_names = all_in_names + [partition_name]

    def _body(*args):
        operands = list(args)
        if partition_name is not None:
            operands.append(bass2jax.partition_id_tensor())
        outs = bass2jax._bass_exec_p.bind(
            *operands,
            out_avals=tuple(out_avals),
            in_names=tuple(all_in_names),
            out_names=tuple(out_names),
            lowering_input_output_aliases=(),
            sim_require_finite=True,
            sim_require_nnan=True,
            nc=nc,
        )
        return tuple(outs)

    devices = jax.devices()[:TILES]
    mesh = Mesh(np.asarray(devices), ("core",))
    n_args = n_params + len(out_names)
    sharded = jax.jit(
        shard_map(
            _body,
            mesh=mesh,
            in_specs=(PartitionSpec("core"),) * n_args,
            out_specs=(PartitionSpec("core"),) * len(out_names),
            check_rep=False,
        ),
        keep_unused=True,
    )
    return sharded, in_names, out_names, out_avals, zero_outs, mesh


def _get_exec_cached(cap: int, mode: str):
    key = (cap, mode)
    if key not in _execs:
        _execs[key] = _build_exec(cap, mode)
    return _execs[key]


def _device_inputs(in_maps, cap, mode):
    """Concat per-core inputs on axis 0 and device_put with the mesh sharding."""
    import jax
    from jax.sharding import NamedSharding, PartitionSpec

    sharded, in_names, out_names, out_avals, zero_outs, mesh = _get_exec_cached(cap, mode)
    spec = NamedSharding(mesh, PartitionSpec("core"))
    args = []
    for name in in_names:
        c = np.concatenate([np.asarray(m[name]) for m in in_maps], axis=0)
        args.append(jax.device_put(c, spec))
    for z in zero_outs:
        c = np.zeros((TILES * z.shape[0], *z.shape[1:]), z.dtype)
        args.append(jax.device_put(c, spec))
    return args


_last_device_args = None
_last_cap = None


def _run_spmd(in_maps, cap, mode):
    global _last_device_args, _last_cap
    sharded, in_names, out_names, out_avals, zero_outs, mesh = _get_exec_cached(cap, mode)
    args = _device_inputs(in_maps, cap, mode)
    _last_device_args, _last_cap = args, cap
    outs = sharded(*args)
    results = []
    for c in range(TILES):
        results.append({
            name: np.asarray(outs[i]).reshape(TILES, *out_avals[i].shape)[c]
            for i, name in enumerate(out_names)
        })
    return results


def _host_reference(xf, up_w, up_b, Wd, bd, tid):
    hidden = np.empty((N, DFF), np.float32)
    for t in range(TILES):
        sel = tid == t
        hidden[sel] = np.maximum(xf[sel] @ up_w[t] + up_b[t], 0.0)
    return hidden @ Wd + bd


def kernel(x, gate_w, gate_b, up_w, up_b, down_w, down_b):
    x = np.asarray(x, np.float32)
    gate_w = np.asarray(gate_w, np.float32)
    gate_b = np.asarray(gate_b, np.float32)
    up_w = np.asarray(up_w, np.float32)
    up_b = np.asarray(up_b, np.float32)
    down_w = np.asarray(down_w, np.float32)
    down_b = np.asarray(down_b, np.float32)

    xf = x.reshape(N, D)
    logits = xf @ gate_w + gate_b
    tid = logits.argmax(1)
    oh = np.zeros((N, TILES), np.float32)
    oh[np.arange(N), tid] = 1.0
    gate = (logits + (oh - logits)).reshape(B, T, TILES)

    Wd = down_w.sum(0)                      # [DFF, D]
    bd = down_b.sum(0)                      # [D]

    idx = [np.nonzero(tid == t)[0] for t in range(TILES)]
    max_count = max(len(i) for i in idx)
    cap = max(256, -(-max_count // P) * P)
    if cap > 512:
        # pathological imbalance; fall back to exact host compute
        out = _host_reference(xf, up_w, up_b, Wd, bd, tid)
        return out.reshape(B, T, D), gate

    mode = MODE
    if mode == "bf16":
        import ml_dtypes
        io_np = ml_dtypes.bfloat16
    else:
        io_np = np.float32

    upb_r = np.ascontiguousarray(up_b.reshape(TILES, DFF // P, P).transpose(0, 2, 1))
    bd_r = np.ascontiguousarray(bd.reshape(D // P, P).T)

    in_maps = []
    for t in range(TILES):
        xT = np.zeros((D, cap), io_np)
        xT[:, :len(idx[t])] = xf[idx[t]].T.astype(io_np)
        in_maps.append({
            "xT": xT,
            "upw": np.ascontiguousarray(up_w[t]).astype(io_np),
            "upb": upb_r[t],
            "wd": Wd.astype(io_np),
            "bd": bd_r,
        })

    results = _run_spmd(in_maps, cap, mode)

    out_flat = np.empty((N, D), np.float32)
    for t in range(TILES):
        out_flat[idx[t]] = results[t]["outT"][:, :len(idx[t])].T
    out = out_flat.reshape(B, T, D)
    return out, gate
